# revision 1
# baseline (speedup 1.0000x reference)
"""GQA causal-attention prefill kernel for Trainium2, tensor-parallel over 8 NeuronCores.

Reference semantics (see problem): q/k/v projections + RoPE + causal GQA
attention + output projection, fp32, B=2, T=2048, D=4096, 32 q heads,
8 kv heads, head_dim 128.

Sharding: head-parallel. Core c gets q heads [4c, 4c+4), kv head c, and the
matching wo slice; each core computes a full-shape partial output
o_part = attn(heads of c) @ wo_c and the host sums the 8 partials
(the tensor-parallel all-reduce, done at unshard time).

Layout strategy on-core (all matmuls fp32r on the PE):
  - x is passed pre-transposed (xT [D, B*T]) so projections contract D on
    the partition dim:  qT/kT/vT[h] = w[h].T @ xT  -> [H=128, tokens].
  - RoPE applied during PSUM eviction (halves of the H partition dim).
  - scores are computed transposed (sT[s, t] = kT_tile.T @ qT) so the
    expensive softmax reduction over s becomes a matmul-side reduction:
    v is stored natural [s, H] with a ones column appended, so
    out_nat[t, 0:128] = sum_s p[s,t] v[s,:] and out_nat[t, 128] = l[t]
    (the softmax denominator) come out of one accumulation group.
  - softmax skips the max-shift (scores/sqrt(H) ~ N(0,1) here, exp is safe
    in fp32); exp is fused into the PSUM eviction on the scalar engine.
  - causal mask = multiply by a 0/1 wedge mask on the diagonal band blocks.
  - normalization folds into the out_nat eviction (per-partition 1/l).
  - out_nat is PE-transposed so the o-projection contracts (h, H) on the
    partition dim against the natural wo layout.
"""

import os
import sys

sys.path.insert(0, "/opt/trn_rl_repo")

import numpy as np

B = 2
T = 2048
TOK = B * T
D = 4096
NQ = 32
NKV = 8
H = 128
HH = H // 2
THETA = 10000.0
NCORES = 8
NHC = NQ // NCORES          # q heads per core (4)
KPC = D // H                # contraction chunks of 128 over D (32)
TCH = 512                   # token chunk for projections / scores free dim
NTCH = T // TCH             # 4 token chunks per batch
C_SM = 1.0 / np.sqrt(H)     # softmax scale


def _build_bass():
    import concourse.bacc as bacc
    import concourse.mybir as mybir
    import concourse.tile as tile
    from concourse.masks import make_identity

    f32 = mybir.dt.float32
    f32r = mybir.dt.float32r
    Exp = mybir.ActivationFunctionType.Exp

    nc = bacc.Bacc("TRN2", target_bir_lowering=False, debug=False,
                   num_devices=NCORES)

    xT = nc.declare_dram_parameter("xT", [D, TOK], f32, isOutput=False)
    wq = nc.declare_dram_parameter("wq", [NHC, D, H], f32, isOutput=False)
    wk = nc.declare_dram_parameter("wk", [D, H], f32, isOutput=False)
    wv = nc.declare_dram_parameter("wv", [D, H], f32, isOutput=False)
    wo = nc.declare_dram_parameter("wo", [NHC, H, D], f32, isOutput=False)
    # rope tables duplicated across both partition halves: row p and row
    # p+64 hold the same values, so every rope operand pair shares a base.
    cosT = nc.declare_dram_parameter("cosT", [H, TOK], f32, isOutput=False)
    sinT = nc.declare_dram_parameter("sinT", [H, TOK], f32, isOutput=False)
    o_part = nc.declare_dram_parameter("o_part", [TOK, D], f32, isOutput=True)

    with tile.TileContext(nc) as tc:
        from contextlib import ExitStack

        with ExitStack() as top:
            # fp32r-consumed constants need their own tensors: the walrus
            # "rounded to FP32r" producer check is tensor-granular.
            consts = top.enter_context(tc.tile_pool(name="consts", bufs=1))
            identity = consts.tile([H, H], f32)
            make_identity(nc, identity)
            ones_f32 = consts.tile([H, 1], f32, tag="ones32")
            nc.vector.memset(ones_f32, 1.0)
            ones_col = consts.tile([H, 1], f32r, tag="ones")
            nc.vector.tensor_copy(ones_col, ones_f32)
            ones_row_f32 = consts.tile([1, H], f32, tag="onesrow32")
            nc.vector.memset(ones_row_f32, 1.0)
            ones_row = consts.tile([1, H], f32r, tag="onesrow")
            nc.vector.tensor_copy(ones_row, ones_row_f32)
            # 0/1 causal wedge masks for the diagonal band:
            # mask[j][s, t] = 1 iff (t - s - 128*j) >= 0
            masks = []
            for j in range(TCH // H):
                m = consts.tile([H, TCH], f32, tag=f"mask{j}",
                                name=f"mask{j}")
                nc.vector.memset(m, 1.0)
                nc.gpsimd.affine_select(
                    out=m, in_=m,
                    compare_op=mybir.AluOpType.is_ge,
                    fill=0.0,
                    base=-H * j,
                    pattern=[[1, TCH]],
                    channel_multiplier=-1,
                )
                masks.append(m)
            for b in range(B):
                tb = b * T
                with ExitStack() as bstk:
                    act = bstk.enter_context(tc.tile_pool(name="act", bufs=1))
                    # activations for this batch (consumed by phase 2), split
                    # per t-chunk: Tile dependency tracking is tile-granular,
                    # so one big tile would make phase 2's first reads wait on
                    # the LAST chunk's eviction tail.
                    qTs = [act.tile([H, NHC, TCH], f32r, tag=f"qT{i}",
                                    name=f"qT{i}") for i in range(NTCH)]
                    kTs = [act.tile([H, TCH], f32r, tag=f"kT{i}",
                                    name=f"kT{i}") for i in range(NTCH)]
                    # v natural: [s within tile, s-tile-within-chunk, H]
                    vs = [act.tile([H, TCH // H, H], f32r, tag=f"v{i}",
                                   name=f"v{i}") for i in range(NTCH)]

                    # phase 1: projections + rope in ONE x-sweep:
                    # 6 accumulation groups (q0-q3, k, v) in 6 PSUM banks plus
                    # 2 transpose banks. Banks are single-buffered; evictions
                    # are staged out via one ACT copy + one DVE half-swap copy
                    # per bank so each bank frees in well under a microsecond,
                    # and the rope math runs on SBUF staging off the critical
                    # path (DVE muls + GpSimd add/sub).
                    with ExitStack() as ph1:
                        wpool = ph1.enter_context(
                            tc.tile_pool(name="wpool", bufs=1))
                        xpool = ph1.enter_context(
                            tc.tile_pool(name="xpool", bufs=4))
                        rtmp = ph1.enter_context(
                            tc.tile_pool(name="rtmp", bufs=2))
                        pj = ph1.enter_context(
                            tc.tile_pool(name="pj", bufs=1, space="PSUM"))
                        pt = ph1.enter_context(
                            tc.tile_pool(name="pt", bufs=2, space="PSUM"))

                        # per-head wq tiles: deps are tile-granular, so the
                        # first matmul of the batch only waits for head 0's
                        # 2MB instead of the whole 8MB load
                        wq_src = (wq.rearrange("h (c p) m -> p h c m", p=H)
                                  .bitcast(f32r))
                        wqs = []
                        for i in range(NHC):
                            wq_h = wpool.tile([H, KPC, H], f32r, tag=f"wq{i}",
                                              name=f"wq{i}")
                            for c8 in range(4):
                                sl = slice(c8 * 8, (c8 + 1) * 8)
                                nc.sync.dma_start(out=wq_h[:, sl, :],
                                                  in_=wq_src[:, i, sl, :])
                            wqs.append(wq_h)
                        wk_sb = wpool.tile([H, KPC, H], f32r, tag="wk")
                        wk_src = (wk.rearrange("(c p) m -> p c m", p=H)
                                  .bitcast(f32r))
                        wv_sb = wpool.tile([H, KPC, H], f32r, tag="wv")
                        wv_src = (wv.rearrange("(c p) m -> p c m", p=H)
                                  .bitcast(f32r))
                        for c16 in range(2):
                            sl = slice(c16 * 16, (c16 + 1) * 16)
                            nc.sync.dma_start(out=wk_sb[:, sl, :],
                                              in_=wk_src[:, sl, :])
                            nc.sync.dma_start(out=wv_sb[:, sl, :],
                                              in_=wv_src[:, sl, :])
                        cos_sb = wpool.tile([H, T], f32, tag="cos")
                        nc.sync.dma_start(out=cos_sb, in_=cosT[:, tb:tb + T])
                        sin_sb = wpool.tile([H, T], f32, tag="sin")
                        nc.sync.dma_start(out=sin_sb, in_=sinT[:, tb:tb + T])

                        def rope_release(psum):
                            # free the PSUM bank fast: ACT copies the bank
                            # straight out, DVE copies it half-swapped; the
                            # rope math later reads SBUF staging only.
                            # All groups' releases are emitted before any math
                            # so no bank release queues behind rope muls on
                            # DVE (per-proc ticks are globally ordered).
                            direct = rtmp.tile([H, TCH], f32, tag="rdir",
                                               bufs=5, name="direct")
                            swap = rtmp.tile([H, TCH], f32, tag="rswap",
                                             bufs=5, name="swap")
                            nc.scalar.activation(
                                direct, psum,
                                mybir.ActivationFunctionType.Copy)
                            nc.vector.tensor_copy(swap[0:HH, :], psum[HH:H, :])
                            nc.vector.tensor_copy(swap[HH:H, :], psum[0:HH, :])
                            return direct, swap

                        def rope_math(direct, swap, dst_first, dst_second,
                                      cs, sn):
                            # (both-SBUF operand pairs must share a base
                            # partition, hence the swapped staging copy.)
                            # All four muls write plain-f32 temps (f32r cast
                            # writes run ~2.4x slower on DVE); GpSimd combines
                            # the products and does the single f32r write, so
                            # each dst has one writer and DVE never waits on
                            # GpSimd.
                            tmp = rtmp.tile([H, TCH], f32, tag="rt", bufs=2)
                            tmp2 = rtmp.tile([H, TCH], f32, tag="rt2", bufs=2)
                            t1 = tmp[0:HH, :]
                            t2 = tmp[HH:H, :]
                            c1 = tmp2[0:HH, :]
                            c2 = tmp2[HH:H, :]
                            nc.vector.tensor_mul(t1, swap[0:HH, :], sn[0:HH, :])
                            nc.vector.tensor_mul(c1, direct[0:HH, :],
                                                 cs[0:HH, :])
                            nc.gpsimd.tensor_sub(dst_first, c1, t1)
                            nc.vector.tensor_mul(t2, swap[HH:H, :], sn[HH:H, :])
                            nc.vector.tensor_mul(c2, direct[HH:H, :],
                                                 cs[HH:H, :])
                            nc.gpsimd.tensor_add(dst_second, c2, t2)

                        last = KPC - 1
                        for tch in range(NTCH):
                            t0 = tch * TCH
                            g_ps = [pj.tile([H, TCH], f32, tag=f"g{i}",
                                            name=f"g_ps{i}")
                                    for i in range(6)]
                            for k in range(KPC):
                                x_t = xpool.tile([H, TCH], f32r, tag="x")
                                nc.sync.dma_start(
                                    out=x_t,
                                    in_=xT[k * H:(k + 1) * H,
                                           tb + t0:tb + t0 + TCH]
                                    .bitcast(f32r))
                                lhs = [wqs[0][:, k, :], wqs[1][:, k, :],
                                       wqs[2][:, k, :], wqs[3][:, k, :],
                                       wk_sb[:, k, :], wv_sb[:, k, :]]
                                for i in range(6):
                                    nc.tensor.matmul(
                                        g_ps[i], lhs[i], x_t,
                                        start=(k == 0), stop=(k == last),
                                        skip_group_check=True)
                            cs = cos_sb[:, t0:t0 + TCH]
                            sn = sin_sb[:, t0:t0 + TCH]
                            # v first: the transposes are the only PE work in
                            # the eviction tail, so emitting them before the
                            # rope chain keeps the tail off the PE's critical
                            # path at the phase boundary.
                            vt_stage = rtmp.tile([H, TCH], f32,
                                                 tag="vstage", bufs=1)
                            nc.vector.tensor_copy(vt_stage, g_ps[5])
                            for j in range(TCH // H):
                                tp = pt.tile([H, H], f32, tag="vtp")
                                nc.tensor.transpose(
                                    tp, vt_stage[:, j * H:(j + 1) * H],
                                    identity)
                                nc.vector.tensor_copy(vs[tch][:, j, :], tp)
                            # release banks in the order the next chunk's
                            # matmuls need them (q0..q3, k); kT's math runs
                            # first since phase 2 consumes kT earliest.
                            rel = [rope_release(g_ps[g]) for g in range(5)]
                            rope_math(*rel[4], kTs[tch][0:HH, :],
                                      kTs[tch][HH:H, :], cs, sn)
                            for i in range(NHC):
                                rope_math(*rel[i], qTs[tch][0:HH, i, :],
                                          qTs[tch][HH:H, i, :], cs, sn)
                    # ---------------- phase 2+3: attention + o-projection --------
                    # Attention per (q-chunk, head), all matmuls with 512-wide
                    # moving operands (fp32r full speed):
                    #   scores:  sT[s-tile, t512] = kT_tile.T @ qT_chunk
                    #   exp (+causal 0/1 mask on the diagonal band) -> pT2
                    #   AV:      avT[H, t512]    += v_tile.T(lhsT=v natural) @ pT2
                    #   denom:   l[1, t512]      += ones.T @ pT2
                    #   normalize: outT = avT * (1/l) broadcast over partitions
                    #              (1/l broadcast via a DRAM roundtrip DMA)
                    with ExitStack() as ph2:
                        # ppool/p2pool first: they should claim addresses in
                        # the early-released weight region, not the
                        # late-released rope staging region
                        ppool = ph2.enter_context(tc.tile_pool(name="ppool", bufs=2))
                        p2pool = ph2.enter_context(tc.tile_pool(name="p2pool", bufs=3))
                        wpool2 = ph2.enter_context(tc.tile_pool(name="wpool2", bufs=1))
                        otpool = ph2.enter_context(tc.tile_pool(name="otpool", bufs=2))
                        small = ph2.enter_context(tc.tile_pool(name="small", bufs=2))
                        opool = ph2.enter_context(tc.tile_pool(name="opool", bufs=2))
                        ps_s = ph2.enter_context(
                            tc.tile_pool(name="ps_s", bufs=2, space="PSUM"))
                        ps_av = ph2.enter_context(
                            tc.tile_pool(name="ps_av", bufs=2, space="PSUM"))
                        ps_l = ph2.enter_context(
                            tc.tile_pool(name="ps_l", bufs=1, space="PSUM"))
                        ps_o = ph2.enter_context(
                            tc.tile_pool(name="ps_o", bufs=2, space="PSUM"))
                        ps_bc = ph2.enter_context(
                            tc.tile_pool(name="ps_bc", bufs=1, space="PSUM"))

                        wo_sb = wpool2.tile([H, NHC, D], f32r)
                        wo_src = wo.rearrange("h p d -> p h d").bitcast(f32r)
                        for dc8 in range(8):
                            sl = slice(dc8 * TCH, (dc8 + 1) * TCH)
                            nc.sync.dma_start(out=wo_sb[:, :, sl],
                                              in_=wo_src[:, :, sl])


                        NSUB = TCH // H  # 4 t-subtiles per q-chunk

                        def emit_oproj(q0_prev, outT_prev):
                            for u in range(NSUB):
                                trow = tb + q0_prev + u * H
                                for dc in range(D // TCH):
                                    ops = ps_o.tile([H, TCH], f32, tag="o")
                                    for h in range(NHC):
                                        nc.tensor.matmul(
                                            ops,
                                            outT_prev[:, h, u * H:(u + 1) * H],
                                            wo_sb[:, h,
                                                  dc * TCH:(dc + 1) * TCH],
                                            start=(h == 0),
                                            stop=(h == NHC - 1),
                                            skip_group_check=True)
                                    o_sb = opool.tile([H, TCH], f32, tag="osb")
                                    nc.scalar.activation(
                                        o_sb, ops,
                                        mybir.ActivationFunctionType.Copy)
                                    nc.sync.dma_start(
                                        out=o_part[trow:trow + H,
                                                   dc * TCH:(dc + 1) * TCH],
                                        in_=o_sb)

                        # o-projection of q-chunk N is emitted after the first
                        # head of q-chunk N+1, hiding the normalize tail.
                        pending = None
                        for qc in range(NTCH):
                            q0 = qc * TCH
                            n_st = (qc + 1) * NSUB
                            outT_sb = otpool.tile([H, NHC, TCH], f32r, tag="outT")
                            for h in range(NHC):
                                rhs_q = qTs[qc][:, h, :]
                                av_ps = ps_av.tile([H, TCH], f32, tag="av")
                                l_ps = ps_l.tile([1, TCH], f32, tag="l")

                                def scores_block(st):
                                    sps = ps_s.tile([H, TCH], f32, tag="s")
                                    kt = kTs[st // NSUB][
                                        :, (st % NSUB) * H:(st % NSUB + 1) * H]
                                    nc.tensor.matmul(sps, kt, rhs_q,
                                                     start=True, stop=True)
                                    pT = ppool.tile([H, TCH], f32, tag="p")
                                    nc.scalar.activation(pT, sps, Exp, scale=C_SM)
                                    pT2 = p2pool.tile([H, TCH], f32r, tag="p2")
                                    j = st - qc * NSUB
                                    if j >= 0:
                                        nc.vector.tensor_mul(pT2, pT, masks[j])
                                    else:
                                        nc.vector.tensor_copy(pT2, pT)
                                    return pT2

                                def av_block(st, pT2):
                                    nc.tensor.matmul(
                                        av_ps, vs[st // NSUB][:, st % NSUB, :],
                                        pT2,
                                        start=(st == 0), stop=(st == n_st - 1),
                                        skip_group_check=True)
                                    nc.tensor.matmul(
                                        l_ps, ones_col, pT2,
                                        start=(st == 0), stop=(st == n_st - 1),
                                        skip_group_check=True)

                                prev = scores_block(0)
                                for st in range(1, n_st):
                                    cur = scores_block(st)
                                    av_block(st - 1, prev)
                                    prev = cur
                                av_block(n_st - 1, prev)

                                # normalize by 1/l: broadcast l across the 128
                                # partitions with a K=1 ones matmul, then a
                                # full-width reciprocal (a [1,512] reciprocal
                                # runs on a single DVE lane, ~6x slower).
                                l_row = small.tile([1, TCH], f32r, tag="lrow")
                                nc.vector.tensor_copy(l_row, l_ps)
                                l_bc = ps_bc.tile([H, TCH], f32, tag="bc")
                                nc.tensor.matmul(l_bc, ones_row, l_row,
                                                 start=True, stop=True)
                                rl_bc = small.tile([H, TCH], f32, tag="rlbc")
                                nc.vector.reciprocal(rl_bc, l_bc)
                                nc.vector.tensor_mul(
                                    outT_sb[:, h, :], av_ps, rl_bc)
                                if h == 0 and pending is not None:
                                    emit_oproj(*pending)
                                    pending = None
                            pending = (q0, outT_sb)
                        emit_oproj(*pending)

    nc.compile()
    return nc


_NC_CACHE = None


def kernel(x, wq, wk, wv, wo, positions):
    global _NC_CACHE
    from concourse.bass_utils import run_bass_kernel_spmd

    x = np.asarray(x, dtype=np.float32)
    wq = np.asarray(wq, dtype=np.float32)
    wk = np.asarray(wk, dtype=np.float32)
    wv = np.asarray(wv, dtype=np.float32)
    wo = np.asarray(wo, dtype=np.float32)
    positions = np.asarray(positions)

    xT = np.ascontiguousarray(x.reshape(TOK, D).T)
    # rope tables, transposed: [H/2, B*T]
    fraction = 2.0 * np.arange(HH, dtype=np.float32) / H
    timescale = (THETA ** fraction).astype(np.float32)
    pos = positions.reshape(TOK).astype(np.float32)
    sinusoid = pos[None, :] / timescale[:, None]
    cosT = np.cos(sinusoid).astype(np.float32)
    sinT = np.sin(sinusoid).astype(np.float32)
    # duplicate across both partition halves (see kernel comment)
    cosT = np.ascontiguousarray(np.concatenate([cosT, cosT], axis=0))
    sinT = np.ascontiguousarray(np.concatenate([sinT, sinT], axis=0))

    if _NC_CACHE is None:
        _NC_CACHE = _build_bass()
    nc = _NC_CACHE

    in_maps = []
    for c in range(NCORES):
        in_maps.append({
            "xT": xT,
            "wq": np.ascontiguousarray(wq[c * NHC:(c + 1) * NHC]),
            "wk": np.ascontiguousarray(wk[c]),
            "wv": np.ascontiguousarray(wv[c]),
            "wo": np.ascontiguousarray(wo[c * NHC:(c + 1) * NHC]),
            "cosT": cosT,
            "sinT": sinT,
        })

    trace = os.environ.get("BASS_KERNEL_TRACE", "0") == "1"
    res = run_bass_kernel_spmd(nc, in_maps, list(range(NCORES)), trace=trace)
    global LAST_RESULTS
    LAST_RESULTS = res
    out = np.zeros((TOK, D), dtype=np.float32)
    for c in range(NCORES):
        out += res.results[c]["o_part"]
    return out.reshape(B, T, D)


LAST_RESULTS = None



# revision 3
# speedup vs baseline: 1.0770x; 1.0770x over previous
"""GQA causal-attention prefill kernel for Trainium2, tensor-parallel over 8 NeuronCores.

v2: all-bf16 datapath, fused single-stream schedule.

Sharding: head-parallel. Core c gets q heads [4c, 4c+4), kv head c, and the
matching wo slice; each core computes a full-shape partial output
o_part = attn(heads of c) @ wo_c and the host sums the 8 partials.

Per-core schedule (one PE stream, no phase barriers):
  for b, c:   PROJ(b,c) k-sweep -> OPROJ(prev chunk) -> ATTN(b,c)
The o-projection of the previous chunk fills the PE while the current
chunk's rope evictions run on DVE, so the PE never waits on the rope tail.

Layouts:
  - xT [D, B*T] bf16; projections contract D on the partition dim:
    qT/kT/vT[h] = w[h].T @ xT -> [H=128, tokens], 6 PSUM groups per chunk.
  - RoPE on eviction, all on DVE reading PSUM directly (4 ops/group):
      tmp[0:64]  = psum[64:128] * sinS[0:64]   (sinS first half = -sin)
      tmp[64:]   = psum[0:64]   * sinS[64:]    (second half = +sin)
      tmp2       = psum * cos
      dst(bf16)  = tmp + tmp2
  - scores transposed (sT[s,t] = kT_tile.T @ qT), exp fused into PSUM
    eviction on ACT (bf16 out); causal diagonal blocks are column-trimmed
    (ap = 512-128j) and masked by a 0/1 wedge multiply on DVE.
  - softmax denominator: lbc[m,t] += ones[128,128].T @ p  (all-ones
    stationary broadcasts the column sum to all 128 partitions, so the
    reciprocal needs no separate broadcast matmul).
  - normalize fused into av eviction: outT[:,h,:] = av_psum * recip(lbc).
  - o-projection contracts (h,H) against natural wo; evictions alternate
    ACT/DVE; output stays f32.
"""

import os
import sys

sys.path.insert(0, "/opt/trn_rl_repo")

import numpy as np

B = 2
T = 2048
TOK = B * T
D = 4096
NQ = 32
NKV = 8
H = 128
HH = H // 2
THETA = 10000.0
NCORES = 8
NHC = NQ // NCORES          # q heads per core (4)
KPC = D // H                # contraction chunks of 128 over D (32)
TCH = 512                   # token chunk
NTCH = T // TCH             # 4 token chunks per batch
NSUB = TCH // H             # 4 128-wide subtiles per chunk
C_SM = 1.0 / np.sqrt(H)     # softmax scale


def _build_bass():
    import concourse.bacc as bacc
    import concourse.mybir as mybir
    import concourse.tile as tile
    from concourse.masks import make_identity
    from contextlib import ExitStack

    f32 = mybir.dt.float32
    bf16 = mybir.dt.bfloat16
    Exp = mybir.ActivationFunctionType.Exp
    Copy = mybir.ActivationFunctionType.Copy

    nc = bacc.Bacc("TRN2", target_bir_lowering=False, debug=False,
                   num_devices=NCORES)

    xT = nc.declare_dram_parameter("xT", [D, TOK], bf16, isOutput=False)
    wq = nc.declare_dram_parameter("wq", [NHC, D, H], bf16, isOutput=False)
    wk = nc.declare_dram_parameter("wk", [D, H], bf16, isOutput=False)
    wv = nc.declare_dram_parameter("wv", [D, H], bf16, isOutput=False)
    wo = nc.declare_dram_parameter("wo", [NHC, H, D], bf16, isOutput=False)
    # rope tables duplicated across both partition halves; sinS's first
    # half is negated so rope reduces to dst = psum*cos + swap(psum)*sinS.
    cosT = nc.declare_dram_parameter("cosT", [H, TOK], f32, isOutput=False)
    sinST = nc.declare_dram_parameter("sinST", [H, TOK], f32, isOutput=False)
    o_part = nc.declare_dram_parameter("o_part", [TOK, D], f32, isOutput=True)

    with tile.TileContext(nc) as tc:
        with ExitStack() as top:
            consts = top.enter_context(tc.tile_pool(name="consts", bufs=1))
            identity = consts.tile([H, H], bf16)
            make_identity(nc, identity)
            ones_sq = consts.tile([H, H], bf16, tag="ones")
            nc.vector.memset(ones_sq, 1.0)
            # 0/1 causal wedge masks: mask[j][s, t] = 1 iff (t - s - 128j) >= 0
            masks = []
            for j in range(NSUB):
                m = consts.tile([H, TCH], bf16, tag=f"mask{j}",
                                name=f"mask{j}")
                nc.vector.memset(m, 1.0)
                nc.gpsimd.affine_select(
                    out=m, in_=m,
                    compare_op=mybir.AluOpType.is_ge,
                    fill=0.0,
                    base=-H * j,
                    pattern=[[1, TCH]],
                    channel_multiplier=-1,
                )
                masks.append(m)

            # ---- persistent weights / tables ----
            wpool = top.enter_context(tc.tile_pool(name="wpool", bufs=1))
            wq_src = wq.rearrange("h (c p) m -> p h c m", p=H)
            wqs = []
            for i in range(NHC):
                wq_h = wpool.tile([H, KPC, H], bf16, tag=f"wq{i}",
                                  name=f"wq{i}")
                for c8 in range(4):
                    sl = slice(c8 * 8, (c8 + 1) * 8)
                    nc.sync.dma_start(out=wq_h[:, sl, :],
                                      in_=wq_src[:, i, sl, :])
                wqs.append(wq_h)
            wk_sb = wpool.tile([H, KPC, H], bf16, tag="wk")
            wk_src = wk.rearrange("(c p) m -> p c m", p=H)
            wv_sb = wpool.tile([H, KPC, H], bf16, tag="wv")
            wv_src = wv.rearrange("(c p) m -> p c m", p=H)
            for c16 in range(2):
                sl = slice(c16 * 16, (c16 + 1) * 16)
                nc.sync.dma_start(out=wk_sb[:, sl, :], in_=wk_src[:, sl, :])
                nc.sync.dma_start(out=wv_sb[:, sl, :], in_=wv_src[:, sl, :])
            cos_sb = wpool.tile([H, TOK], f32, tag="cos")
            sin_sb = wpool.tile([H, TOK], f32, tag="sin")
            for c4 in range(4):
                sl = slice(c4 * 1024, (c4 + 1) * 1024)
                nc.sync.dma_start(out=cos_sb[:, sl], in_=cosT[:, sl])
                nc.sync.dma_start(out=sin_sb[:, sl], in_=sinST[:, sl])
            wo_sb = wpool.tile([H, NHC, D], bf16, tag="wo")
            wo_src = wo.rearrange("h p d -> p h d")
            for dc8 in range(8):
                sl = slice(dc8 * TCH, (dc8 + 1) * TCH)
                nc.sync.dma_start(out=wo_sb[:, :, sl], in_=wo_src[:, :, sl])

            # ---- persistent activations ----
            act = top.enter_context(tc.tile_pool(name="act", bufs=1))
            qTs = [act.tile([H, NHC, TCH], bf16, tag=f"qT{i}",
                            name=f"qT{i}") for i in range(NTCH)]
            kTs = [act.tile([H, TCH], bf16, tag=f"kT{i}",
                            name=f"kT{i}") for i in range(NTCH)]
            vs = [act.tile([H, NSUB, H], bf16, tag=f"v{i}",
                           name=f"v{i}") for i in range(NTCH)]

            xpool = top.enter_context(tc.tile_pool(name="xpool", bufs=12))
            rtmp = top.enter_context(tc.tile_pool(name="rtmp", bufs=2))
            vstg = top.enter_context(tc.tile_pool(name="vstg", bufs=2))
            ppool = top.enter_context(tc.tile_pool(name="ppool", bufs=4))
            pdiag = top.enter_context(tc.tile_pool(name="pdiag", bufs=2))
            rlpool = top.enter_context(tc.tile_pool(name="rlpool", bufs=2))
            otpool = top.enter_context(tc.tile_pool(name="otpool", bufs=2))
            opool = top.enter_context(tc.tile_pool(name="opool", bufs=4))

            def rope(psum, dst, cs, sn):
                tmp = rtmp.tile([H, TCH], f32, tag="rt")
                tmp2 = rtmp.tile([H, TCH], f32, tag="rt2")
                nc.vector.tensor_mul(tmp[0:HH, :], psum[HH:H, :], sn[0:HH, :])
                nc.vector.tensor_mul(tmp[HH:H, :], psum[0:HH, :], sn[HH:H, :])
                nc.vector.tensor_mul(tmp2, psum, cs)
                nc.vector.tensor_add(dst, tmp, tmp2)

            def emit_oproj(pend, po):
                pb, pc, outT = pend
                for u in range(NSUB):
                    trow = pb * T + pc * TCH + u * H
                    for dc in range(D // TCH):
                        ops = po.tile([H, TCH], f32, tag="o")
                        for h in range(NHC):
                            nc.tensor.matmul(
                                ops,
                                outT[:, h, u * H:(u + 1) * H],
                                wo_sb[:, h, dc * TCH:(dc + 1) * TCH],
                                start=(h == 0), stop=(h == NHC - 1),
                                skip_group_check=True)
                        o_sb = opool.tile([H, TCH], f32, tag="osb")
                        if (u * 8 + dc) % 2 == 0:
                            nc.scalar.activation(o_sb, ops, Copy)
                        else:
                            nc.vector.tensor_copy(o_sb, ops)
                        nc.sync.dma_start(
                            out=o_part[trow:trow + H,
                                       dc * TCH:(dc + 1) * TCH],
                            in_=o_sb)

            pending = None
            for b in range(B):
                tb = b * T
                for c in range(NTCH):
                    t0 = tb + c * TCH
                    cs = cos_sb[:, t0:t0 + TCH]
                    sn = sin_sb[:, t0:t0 + TCH]
                    with ExitStack() as s1:
                        pj = s1.enter_context(
                            tc.tile_pool(name="pj", bufs=1, space="PSUM"))
                        po = s1.enter_context(
                            tc.tile_pool(name="po", bufs=2, space="PSUM"))
                        g = [pj.tile([H, TCH], f32, tag=f"g{i}",
                                     name=f"g{i}") for i in range(6)]
                        for k in range(KPC):
                            x_t = xpool.tile([H, TCH], bf16, tag="x")
                            nc.sync.dma_start(
                                out=x_t,
                                in_=xT[k * H:(k + 1) * H, t0:t0 + TCH])
                            lhs = [wqs[0][:, k, :], wqs[1][:, k, :],
                                   wqs[2][:, k, :], wqs[3][:, k, :],
                                   wk_sb[:, k, :], wv_sb[:, k, :]]
                            for i in range(6):
                                nc.tensor.matmul(
                                    g[i], lhs[i], x_t,
                                    start=(k == 0), stop=(k == KPC - 1),
                                    skip_group_check=True)
                        # o-projection of the previous chunk: 27us of PE
                        # work covering the rope evictions below (DVE).
                        if pending is not None:
                            emit_oproj(pending, po)
                            pending = None
                        # evictions: v staging (ACT), rope (DVE)
                        vt_stage = vstg.tile([H, TCH], bf16, tag="vstage")
                        nc.scalar.activation(vt_stage, g[5], Copy)
                        rope(g[0], qTs[c][:, 0, :], cs, sn)
                        rope(g[4], kTs[c], cs, sn)
                        rope(g[1], qTs[c][:, 1, :], cs, sn)
                        rope(g[2], qTs[c][:, 2, :], cs, sn)
                        rope(g[3], qTs[c][:, 3, :], cs, sn)
                    with ExitStack() as s2:
                        pt = s2.enter_context(
                            tc.tile_pool(name="pt", bufs=2, space="PSUM"))
                        ps_s = s2.enter_context(
                            tc.tile_pool(name="ps_s", bufs=2, space="PSUM"))
                        ps_av = s2.enter_context(
                            tc.tile_pool(name="ps_av", bufs=2, space="PSUM"))
                        ps_l = s2.enter_context(
                            tc.tile_pool(name="ps_l", bufs=2, space="PSUM"))
                        for j in range(NSUB):
                            tp = pt.tile([H, H], bf16, tag="vtp")
                            nc.tensor.transpose(
                                tp, vt_stage[:, j * H:(j + 1) * H], identity)
                            nc.scalar.activation(vs[c][:, j, :], tp, Copy)
                        n_st = (c + 1) * NSUB
                        outT_sb = otpool.tile([H, NHC, TCH], bf16, tag="outT")
                        for h in range(NHC):
                            av_ps = ps_av.tile([H, TCH], f32, tag="av")
                            l_ps = ps_l.tile([H, TCH], f32, tag="lbc")
                            for st in range(n_st):
                                j = st - c * NSUB
                                off = H * j if j > 0 else 0
                                kt = kTs[st // NSUB][
                                    :, (st % NSUB) * H:(st % NSUB + 1) * H]
                                sps = ps_s.tile([H, TCH], f32, tag="s")
                                nc.tensor.matmul(
                                    sps[:, off:], kt, qTs[c][:, h, off:],
                                    start=True, stop=True)
                                pT2 = ppool.tile([H, TCH], bf16, tag="p2")
                                if j >= 0:
                                    pT = pdiag.tile([H, TCH], bf16, tag="pd")
                                    nc.scalar.activation(
                                        pT[:, off:], sps[:, off:], Exp,
                                        scale=C_SM)
                                    nc.vector.tensor_mul(
                                        pT2[:, off:], pT[:, off:],
                                        masks[j][:, off:])
                                else:
                                    nc.scalar.activation(
                                        pT2, sps, Exp, scale=C_SM)
                                nc.tensor.matmul(
                                    av_ps[:, off:],
                                    vs[st // NSUB][:, st % NSUB, :],
                                    pT2[:, off:],
                                    start=(st == 0), stop=(st == n_st - 1),
                                    skip_group_check=True)
                                nc.tensor.matmul(
                                    l_ps[:, off:], ones_sq, pT2[:, off:],
                                    start=(st == 0), stop=(st == n_st - 1),
                                    skip_group_check=True)
                            rl = rlpool.tile([H, TCH], f32, tag="rl")
                            nc.vector.reciprocal(rl, l_ps)
                            nc.vector.tensor_mul(outT_sb[:, h, :], av_ps, rl)
                        pending = (b, c, outT_sb)
            with ExitStack() as s3:
                po = s3.enter_context(
                    tc.tile_pool(name="po3", bufs=2, space="PSUM"))
                emit_oproj(pending, po)

    nc.compile()
    return nc


_NC_CACHE = None


def kernel(x, wq, wk, wv, wo, positions):
    global _NC_CACHE
    import ml_dtypes
    from concourse.bass_utils import run_bass_kernel_spmd

    bf16 = ml_dtypes.bfloat16
    x = np.asarray(x, dtype=np.float32)
    wq = np.asarray(wq, dtype=np.float32)
    wk = np.asarray(wk, dtype=np.float32)
    wv = np.asarray(wv, dtype=np.float32)
    wo = np.asarray(wo, dtype=np.float32)
    positions = np.asarray(positions)

    xT = np.ascontiguousarray(x.reshape(TOK, D).T).astype(bf16)
    # rope tables, [H, B*T]: duplicated across partition halves; sin's
    # first half negated (see kernel docstring).
    fraction = 2.0 * np.arange(HH, dtype=np.float32) / H
    timescale = (THETA ** fraction).astype(np.float32)
    pos = positions.reshape(TOK).astype(np.float32)
    sinusoid = pos[None, :] / timescale[:, None]
    cos_h = np.cos(sinusoid).astype(np.float32)
    sin_h = np.sin(sinusoid).astype(np.float32)
    cosT = np.ascontiguousarray(np.concatenate([cos_h, cos_h], axis=0))
    sinST = np.ascontiguousarray(np.concatenate([-sin_h, sin_h], axis=0))

    if _NC_CACHE is None:
        _NC_CACHE = _build_bass()
    nc = _NC_CACHE

    in_maps = []
    for c in range(NCORES):
        in_maps.append({
            "xT": xT,
            "wq": np.ascontiguousarray(wq[c * NHC:(c + 1) * NHC]).astype(bf16),
            "wk": np.ascontiguousarray(wk[c]).astype(bf16),
            "wv": np.ascontiguousarray(wv[c]).astype(bf16),
            "wo": np.ascontiguousarray(wo[c * NHC:(c + 1) * NHC]).astype(bf16),
            "cosT": cosT,
            "sinST": sinST,
        })

    trace = os.environ.get("BASS_KERNEL_TRACE", "0") == "1"
    res = run_bass_kernel_spmd(nc, in_maps, list(range(NCORES)), trace=trace)
    global LAST_RESULTS
    LAST_RESULTS = res
    out = np.zeros((TOK, D), dtype=np.float32)
    for c in range(NCORES):
        out += res.results[c]["o_part"]
    return out.reshape(B, T, D)


LAST_RESULTS = None
